# revision 17
# baseline (speedup 1.0000x reference)
"""Context-Query attention (BiDAF-style trilinear attention + dual softmax)
for Trainium2, data-parallel over batch across 8 NeuronCores.

Math (per batch b, all masks are ones and bias cancels in both softmaxes):
  Ct = C^T [Lc,d], Qt = Q^T [Lq,d]
  S = s0[c] + s1[q] + s2[c,q],  s2 = Ct.diag(w4mlu).Qt^T
  S1 = softmax_q(S) = P1 / rowsum,  P1 = exp(s2 + s1[q])      (s0 cancels)
  S2 = softmax_c(S) = P2 / colsum,  P2 = exp(s2 + s0[c])      (s1 cancels)
  A  = S1 @ Qt
  Bm = S1 @ (S2^T @ Ct)
  out = concat([Ct, A, Ct*A, Ct*Bm], axis=-1)^T  -> [4d, Lc]

Kernel strategy per core (4 batches), tuned against the instruction cost
model timeline:
  - s2 on PE in both orientations (cheaper than transposing S).
  - exp on ACT with per-partition bias columns; P2/P1T/Ct/Qt/Tpp in bf16
    (relative softmax-weight precision is what matters; halves SBUF).
  - all PE transposes stream a bf16 identity (cost model keys the
    cycles/row on the moving identity operand: 1.0 c/r).
  - ones columns on CtOnes/Tpp make colsum/rowsum fall out of the T and
    B matmuls for free.
  - outputs produced per 512-column group so stores stream and the tail
    after the last matmul is short; next batch loads prefetched early.
  - engine split: ACT = exps + A-norms + small copies, DVE = Ct copies +
    B-norms + products, POOL = transpose psum drains.
"""

import os
import sys

sys.path.insert(0, "/opt/trn_rl_repo")

import numpy as np

import concourse.bass as bass
import concourse.bacc as bacc
import concourse.mybir as mybir
from concourse import tile
from concourse.bass_utils import run_bass_kernel_spmd

F32 = mybir.dt.float32
F32R = mybir.dt.float32r
BF16 = mybir.dt.bfloat16
EXP = mybir.ActivationFunctionType.Exp
P = 128

B, D, LC, LQ = 32, 256, 2048, 512
NCORES = 8
BPC = B // NCORES          # batches per core
KD = D // P                # 2 k-tiles over d
NCT = LC // P              # 16 c-tiles
NQT = LQ // P              # 4 q-tiles
NG = 4                     # output column groups of 512


def _body(nc, tc, Cin, Qin, Out, identf_dram, w4c_dram, w4q_dram, mlu_dram):
    ctx_pools = []

    def pool(name, **kw):
        p = tc.tile_pool(name=name, **kw)
        ctx_pools.append(p)
        return p.__enter__()

    const = pool("const", bufs=1)
    sb = pool("sb", bufs=1)
    ps = pool("ps", bufs=1, space=bass.MemorySpace.PSUM)

    identr = const.tile([P, P], F32R, tag="identr", name="identr")
    ident = const.tile([P, P], BF16, tag="ident", name="ident")
    w4c = const.tile([P, KD], F32, tag="w4c", name="w4c")
    w4q = const.tile([P, KD], F32, tag="w4q", name="w4q")
    mlu = const.tile([P, KD], F32, tag="mlu", name="mlu")
    ones1 = const.tile([P, 2], BF16, tag="ones1", name="ones1")

    def emit_consts():
        nc.sync.dma_start(mlu[:], mlu_dram.ap().rearrange("a b (k p) -> p (a b k)", p=P))
        nc.sync.dma_start(w4q[:], w4q_dram.ap().rearrange("(k p) o -> p (k o)", p=P))
        nc.sync.dma_start(w4c[:], w4c_dram.ap().rearrange("(k p) o -> p (k o)", p=P))
        nc.sync.dma_start(identr[:], identf_dram.ap().bitcast(F32R))
        nc.scalar.copy(ident[:], identr[:].bitcast(F32))
        nc.vector.memset(ones1[:], 1.0)

    Cs = {}
    Qs = {}

    def emit_loads(b):
        Qs[b] = []
        for k in range(KD):
            t = sb.tile([P, LQ], F32R, tag=f"Q{k}", name=f"Q{k}_{b}", bufs=3)
            nc.sync.dma_start(t[:], Qin.ap()[b, k * P:(k + 1) * P, :].bitcast(F32R))
            Qs[b].append(t)
        Cs[b] = []
        for k in range(KD):
            t = sb.tile([P, LC], F32R, tag=f"C{k}", name=f"C{k}_{b}", bufs=3)
            Cs[b].append(t)
        # quarter-width chunks so the first c-tiles land early
        for q in range(4):
            for k in range(KD):
                nc.sync.dma_start(
                    Cs[b][k][:, q * 512:(q + 1) * 512],
                    Cin.ap()[b, k * P:(k + 1) * P, q * 512:(q + 1) * 512].bitcast(F32R),
                )

    Qs[0] = []
    for k in range(KD):
        t = sb.tile([P, LQ], F32R, tag=f"Q{k}", name=f"Q{k}_0", bufs=3)
        Qs[0].append(t)
    Cs[0] = []
    for k in range(KD):
        t = sb.tile([P, LC], F32R, tag=f"C{k}", name=f"C{k}_0", bufs=3)
        Cs[0].append(t)
    nc.sync.dma_start(Qs[0][0][:], Qin.ap()[0, 0:P, :].bitcast(F32R))
    for k in range(KD):
        nc.sync.dma_start(
            Cs[0][k][:, 0:512], Cin.ap()[0, k * P:(k + 1) * P, 0:512].bitcast(F32R),
        )
    emit_consts()
    nc.sync.dma_start(Qs[0][1][:], Qin.ap()[0, P:2 * P, :].bitcast(F32R))
    for q in range(1, 4):
        for k in range(KD):
            nc.sync.dma_start(
                Cs[0][k][:, q * 512:(q + 1) * 512],
                Cin.ap()[0, k * P:(k + 1) * P, q * 512:(q + 1) * 512].bitcast(F32R),
            )

    for b in range(BPC):
        C_sb = Cs[b]
        Q_sb = Qs[b]

        # ---- pass-through Ct output block; then prefetch next batch ----
        for h in range(KD):
            nc.sync.dma_start(Out.ap()[b, h * P:(h + 1) * P, :], C_sb[h][:].bitcast(F32))
        if b + 1 < BPC:
            emit_loads(b + 1)

        # ---- Qp = Q * w4mlu (per-partition over d) ----
        Qp = []
        for k in range(KD):
            t = sb.tile([P, LQ], F32R, tag=f"Qp{k}", name=f"Qp{k}_{b}", bufs=2)
            nc.vector.tensor_scalar_mul(t[:], Q_sb[k][:].bitcast(F32), mlu[:, k:k + 1])
            Qp.append(t)

        # ---- s1 similarity columns (tiny matmuls into one bank) ----
        rs = ps.tile([P, 512], F32, tag="rs", name=f"rs_{b}", bufs=1)
        ps01 = rs
        for j in range(NQT):
            for k in range(KD):
                nc.tensor.matmul(
                    ps01[:, 16 + j:17 + j], Q_sb[k][:, j * P:(j + 1) * P].bitcast(F32),
                    w4q[:, k:k + 1], start=(k == 0), stop=(k == KD - 1),
                )
        s1sb = sb.tile([P, NQT], F32, tag="s1", name=f"s1_{b}", bufs=2)
        nc.scalar.copy(s1sb[:], ps01[:, 16:20])

        # ---- P1T[j] = exp(s2_qc + s1[q]) in bf16, chunk-major over c so each
        # unit only needs the C quarter that has already landed; s0 columns,
        # es0 = exp(s0), and the Qt transposes ride along in matching units ----
        P1T = [
            sb.tile([P, LC], BF16, tag=f"P1T_{j}", name=f"P1T_{b}_{j}")
            for j in range(NQT)
        ]
        Qt = [None] * NQT
        CtO = [None] * NCT
        es0 = None
        ptrQ = None
        for n in range(NG):
            for j in range(NQT):
                if j == 0:
                    # s0 columns for the c-tiles covered by this C quarter
                    for i in range(4 * n, 4 * n + 4):
                        for k in range(KD):
                            nc.tensor.matmul(
                                ps01[:, i:i + 1], C_sb[k][:, i * P:(i + 1) * P].bitcast(F32),
                                w4c[:, k:k + 1], start=(k == 0), stop=(k == KD - 1),
                            )
                    if n == NG - 1:
                        es0 = sb.tile([P, NCT], F32, tag="es0", name=f"es0_{b}", bufs=2)
                        nc.scalar.activation(es0[:], ps01[:, 0:NCT], EXP)
                acc = ps.tile([P, 512], F32, tag="w", name=f"psB_{b}_{j}_{n}", bufs=3)
                for k in range(KD):
                    nc.tensor.matmul(
                        acc[:], Qp[k][:, j * P:(j + 1) * P],
                        C_sb[k][:, n * 512:(n + 1) * 512],
                        start=(k == 0), stop=(k == KD - 1),
                    )
                nc.scalar.activation(
                    P1T[j][:, n * 512:(n + 1) * 512], acc[:], EXP, bias=s1sb[:, j:j + 1],
                )
                if j in (1, 3):
                    base_i = 4 * n + (0 if j == 1 else 2)
                    ptrC = ps.tile([P, 512], F32R, tag="w", name=f"ptrC_{b}_{base_i}", bufs=3)
                    for jj in range(2):
                        ii = base_i + jj
                        for k in range(KD):
                            nc.tensor.transpose(
                                ptrC[:, jj * 256 + k * P:jj * 256 + (k + 1) * P],
                                C_sb[k][:, ii * P:(ii + 1) * P], identr[:],
                            )
                    for jj in range(2):
                        ii = base_i + jj
                        tc_ = sb.tile([P, D + 2], BF16, tag=f"Ct_{ii}", name=f"Ct_{b}_{ii}")
                        nc.vector.tensor_copy(tc_[:, 0:D], ptrC[:, jj * 256:jj * 256 + 256].bitcast(F32))
                        nc.vector.memset(tc_[:, D:D + 2], 1.0)
                        CtO[ii] = tc_
                if n == NG - 1 and j in (1, 3):
                    # Qt transposes once ps01 is freed (after es0)
                    ptrQ = ps.tile([P, 512], F32R, tag="ab", name=f"ptrQ_{b}_{j}", bufs=2)
                    for jj in range(2):
                        jq = (j - 1) + jj
                        for k in range(KD):
                            nc.tensor.transpose(
                                ptrQ[:, jj * 256 + k * P: jj * 256 + (k + 1) * P],
                                Q_sb[k][:, jq * P:(jq + 1) * P], identr[:],
                            )
                    for jj in range(2):
                        jq = (j - 1) + jj
                        t = sb.tile([P, 2 * D], BF16, tag=f"QTp{jq}", name=f"QTp{jq}_{b}", bufs=2)
                        nc.vector.tensor_copy(t[:, 0:D], ptrQ[:, jj * 256:jj * 256 + 256].bitcast(F32))
                        Qt[jq] = t

        # ---- P2'[i] = transpose(P1T)*e^{s0[c]}; T accumulation interleaved ----
        # S2 = P2'/colsum is exact: the e^{s1[q]} factor in P2' cancels in the
        # column softmax, and e^{s0[c]} enters per-partition here.
        psT = []
        for j in range(NQT):
            if j < 2:
                t = ps.tile([P, 512], F32, tag="ab", name=f"psT_{b}_{j}", bufs=2)
                psT.append(t)
            else:
                t = ps.tile([P, 1024], BF16, tag="wtr", name=f"psT_{b}_{j}", bufs=2)
                psT.append(t[:].bitcast(F32))
        P2p = [None] * NCT
        LAG = 2

        def T_unit(i):
            for j in range(NQT):
                nc.tensor.matmul(
                    psT[j][:, 0:D + 2], P2p[i][:, j * P:(j + 1) * P], CtO[i][:],
                    start=(i == 0), stop=(i == NCT - 1),
                )

        for i in range(NCT):
            trp2 = ps.tile([P, 512], F32, tag="w", name=f"trp2_{b}_{i}", bufs=3)
            v2 = trp2[:].bitcast(BF16)
            for j in range(NQT):
                nc.tensor.transpose(
                    v2[:, j * P:(j + 1) * P], P1T[j][:, i * P:(i + 1) * P], ident[:],
                )
            tp = sb.tile([P, LQ], BF16, tag=f"P2_{i % 4}", name=f"P2_{b}_{i}", bufs=2)
            if i % 2 == 0:
                nc.scalar.mul(tp[:], v2[:, 0:512], es0[:, i:i + 1])
            else:
                nc.vector.tensor_scalar_mul(tp[:], v2[:, 0:512], es0[:, i:i + 1])
            P2p[i] = tp
            if i >= LAG:
                T_unit(i - LAG)
        for i in range(NCT - LAG, NCT):
            T_unit(i)
        for j in range(NQT):
            cinv = sb.tile([P, 1], F32, tag="cinv", name=f"cinv_{b}_{j}", bufs=2)
            nc.vector.reciprocal(cinv[:], psT[j][:, D:D + 1])
            nc.vector.tensor_scalar_mul(Qt[j][:, D:2 * D], psT[j][:, 0:D], cinv[:])

        # ---- A/B phase per c-tile, output flushed per half-group of 2 ----
        AB_sb = [None] * NCT

        def flush(g):
            c0 = g * 512
            for h in range(KD):
                tra = ps.tile([P, 1024], BF16, tag="wtr", name=f"trA_{b}_{g}_{h}", bufs=2)
                for u in range(4):
                    nc.tensor.transpose(
                        tra[:, u * P:(u + 1) * P],
                        AB_sb[4 * g + u][:, h * P:(h + 1) * P], ident[:],
                    )
                tsb = sb.tile([P, 512], F32, tag=f"trs{h}", name=f"tsbA_{b}_{g}_{h}", bufs=2)
                nc.scalar.copy(tsb[:], tra[:, 0:512])
                nc.sync.dma_start(
                    Out.ap()[b, D + h * P:D + (h + 1) * P, c0:c0 + 512],
                    tsb[:],
                )
                proda = sb.tile([P, 512], F32, tag=f"prA{h}", name=f"prA_{b}_{g}_{h}", bufs=2)
                nc.gpsimd.tensor_mul(
                    proda[:], C_sb[h][:, c0:c0 + 512].bitcast(F32), tsb[:]
                )
                nc.sync.dma_start(
                    Out.ap()[b, 2 * D + h * P:2 * D + (h + 1) * P, c0:c0 + 512],
                    proda[:],
                )
                trb = ps.tile([P, 1024], BF16, tag="wtr", name=f"trB_{b}_{g}_{h}", bufs=2)
                for u in range(4):
                    nc.tensor.transpose(
                        trb[:, u * P:(u + 1) * P],
                        AB_sb[4 * g + u][:, D + h * P:D + (h + 1) * P], ident[:],
                    )
                prodb = sb.tile([P, 512], F32, tag=f"prB{h}", name=f"prB_{b}_{g}_{h}", bufs=2)
                nc.vector.tensor_mul(
                    prodb[:], C_sb[h][:, c0:c0 + 512].bitcast(F32), trb[:, 0:512]
                )
                nc.sync.dma_start(
                    Out.ap()[b, 3 * D + h * P:3 * D + (h + 1) * P, c0:c0 + 512],
                    prodb[:],
                )

        for i in range(NCT):
            for j in range(NQT):
                nc.tensor.matmul(
                    rs[:, 32 + i:33 + i], P1T[j][:, i * P:(i + 1) * P], ones1[:, 0:1],
                    start=(j == 0), stop=(j == NQT - 1),
                )
            acc = ps.tile([P, 512], F32, tag="ab", name=f"psAB_{b}_{i}", bufs=2)
            for j in range(NQT):
                nc.tensor.matmul(
                    acc[:], P1T[j][:, i * P:(i + 1) * P], Qt[j][:],
                    start=(j == 0), stop=(j == NQT - 1),
                )
            rinv = sb.tile([P, 1], F32, tag=f"rinv{i % 4}", name=f"rinv_{b}_{i}", bufs=2)
            nc.vector.reciprocal(rinv[:], rs[:, 32 + i:33 + i])
            tab = sb.tile([P, 2 * D], BF16, tag=f"ABsb{i % 8}", name=f"ABsb_{b}_{i}")
            nc.scalar.mul(tab[:], acc[:], rinv[:])
            AB_sb[i] = tab
            if i >= 5 and (i - 5) % 4 == 0:
                flush((i - 5) // 4)
        if b < BPC - 1:
            flush(NG - 1)
        else:
            for hf in (6, 7):
                c0 = hf * 256
                for h in range(KD):
                    tra = ps.tile([P, 1024], BF16, tag="wtr", name=f"ftrA_{hf}_{h}", bufs=2)
                    for u in range(2):
                        nc.tensor.transpose(
                            tra[:, u * P:(u + 1) * P],
                            AB_sb[2 * hf + u][:, h * P:(h + 1) * P], ident[:],
                        )
                    tsb = sb.tile([P, 256], F32, tag=f"ftrs{h}", name=f"ftsbA_{hf}_{h}", bufs=2)
                    nc.scalar.copy(tsb[:], tra[:, 0:256])
                    nc.sync.dma_start(
                        Out.ap()[b, D + h * P:D + (h + 1) * P, c0:c0 + 256], tsb[:],
                    )
                    proda = sb.tile([P, 256], F32, tag=f"fprA{h}", name=f"fprA_{hf}_{h}", bufs=2)
                    nc.vector.tensor_mul(
                        proda[:], C_sb[h][:, c0:c0 + 256].bitcast(F32), tsb[:]
                    )
                    nc.sync.dma_start(
                        Out.ap()[b, 2 * D + h * P:2 * D + (h + 1) * P, c0:c0 + 256], proda[:],
                    )
                    trb = ps.tile([P, 1024], BF16, tag="wtr", name=f"ftrB_{hf}_{h}", bufs=2)
                    for u in range(2):
                        nc.tensor.transpose(
                            trb[:, u * P:(u + 1) * P],
                            AB_sb[2 * hf + u][:, D + h * P:D + (h + 1) * P], ident[:],
                        )
                    prodb = sb.tile([P, 256], F32, tag=f"fprB{h}", name=f"fprB_{hf}_{h}", bufs=2)
                    nc.vector.tensor_mul(
                        prodb[:], C_sb[h][:, c0:c0 + 256].bitcast(F32), trb[:, 0:256]
                    )
                    nc.sync.dma_start(
                        Out.ap()[b, 3 * D + h * P:3 * D + (h + 1) * P, c0:c0 + 256], prodb[:],
                    )

    for p in reversed(ctx_pools):
        p.__exit__(None, None, None)


def build_nc():
    nc = bacc.Bacc("TRN2", target_bir_lowering=False, debug=False, num_devices=NCORES)
    Cin = nc.dram_tensor("C", [BPC, D, LC], F32, kind="ExternalInput")
    Qin = nc.dram_tensor("Q", [BPC, D, LQ], F32, kind="ExternalInput")
    w4c_dram = nc.dram_tensor("w4C", [D, 1], F32, kind="ExternalInput")
    w4q_dram = nc.dram_tensor("w4Q", [D, 1], F32, kind="ExternalInput")
    mlu_dram = nc.dram_tensor("w4mlu", [1, 1, D], F32, kind="ExternalInput")
    Out = nc.dram_tensor("out", [BPC, 4 * D, LC], F32, kind="ExternalOutput")
    identf_dram = nc.inline_tensor(np.eye(P, dtype=np.float32), name="ident_c")
    with tile.TileContext(nc) as tc:
        _body(nc, tc, Cin, Qin, Out, identf_dram, w4c_dram, w4q_dram, mlu_dram)
    nc.compile()
    return nc


_NC_CACHE = None


def kernel(**inputs):
    global _NC_CACHE
    C = np.ascontiguousarray(np.asarray(inputs["C"], dtype=np.float32))
    Q = np.ascontiguousarray(np.asarray(inputs["Q"], dtype=np.float32))
    w4C = np.ascontiguousarray(np.asarray(inputs["w4C"], dtype=np.float32))
    w4Q = np.ascontiguousarray(np.asarray(inputs["w4Q"], dtype=np.float32))
    w4mlu = np.ascontiguousarray(np.asarray(inputs["w4mlu"], dtype=np.float32))
    # Cmask/Qmask are all-ones and `bias` cancels in both softmaxes -> unused.

    if _NC_CACHE is None:
        _NC_CACHE = build_nc()
    nc = _NC_CACHE
    in_maps = [
        {
            "C": C[i * BPC:(i + 1) * BPC],
            "Q": Q[i * BPC:(i + 1) * BPC],
            "w4C": w4C,
            "w4Q": w4Q,
            "w4mlu": w4mlu,
        }
        for i in range(NCORES)
    ]
    res = run_bass_kernel_spmd(nc, in_maps, list(range(NCORES)))
    out = np.concatenate([res.results[i]["out"] for i in range(NCORES)], axis=0)
    return out
